# revision 47
# baseline (speedup 1.0000x reference)
"""Causal self-attention Trainium2 kernel (8-core head-parallel tensor parallel).

v5 strategy (per core, 2 heads, feature-major dataflow; all-bf16 matmul
operands -- fp8 fails the 2e-2 gate on this data, fp32 moving operands
cost 2x on the PE's moving-data path):
  - QKV: qkv^T = W^T.T @ x^T per 512-token chunk; x and W are host-cast
    to bf16 so each matmul streams at the 216ns/512-col floor. v (m=2)
    is computed FIRST in each chunk so its DVE bias-evict and the
    dependent V transposes never gate the next group's diagonal AV.
    Chunk 0 runs up front; later chunks ride as PE filler inside the
    attention stream.
  - Attention per (b, q-chunk) group, k-tile loop with BOTH heads per
    step: S pair into one 2-bank PSUM tile -> ONE exp on ACT per step
    -> causal affine_select on GpSimd for diagonal tiles. Diagonal
    k-tiles use causal RECTANGLES: S/exp/mask/AV all skip the fully
    masked q < 128*di slice (saves ~20% of attention work and exp).
  - Z rides as a ones-column in V (AV row 64). Normalize is DEFERRED:
    pushed into the next group's k-loop via the filler queue (Z copy on
    ACT, reciprocal_approx_fast on DVE, per-head partition_broadcast on
    GpSimd + multiply on DVE), so the serial chain never blocks the
    v-evict -> vtrans -> S critical path at group boundaries.
  - PE never idles: a filler queue (later QKV chunks, V transposes) and
    a deferred queue (norm + projection of completed groups) are popped
    every k-tile step; one projection quad is held back for the final
    group's norm window. Projection output DMAs are 4-chunk batched.
  - Host: sum 8 bf16 partial out^T in f32, transpose, +b_proj.
"""

import sys

if "/opt/trn_rl_repo" not in sys.path:
    sys.path.insert(0, "/opt/trn_rl_repo")

import numpy as np

# ---- problem constants (hardcoded for the grading harness) ----
B, T, C, H = 2, 2048, 1024, 16
HD = C // H            # 64
N_CORES = 8
HPC = H // N_CORES     # heads per core = 2

_F32R = True


def _cfg_full():
    return dict(B=B, T=T, C=C, HPC=HPC, f32r=_F32R)


MM_LOG = []


def build_nc(cfg):
    """Build the single-core SPMD Bass program."""
    import concourse.bacc as bacc
    import concourse.mybir as mybir
    import concourse.tile as tile
    from concourse.masks import make_identity

    MM_LOG.clear()

    Bc, Tc, Cc, hpc = cfg["B"], cfg["T"], cfg["C"], cfg["HPC"]
    f32r = mybir.dt.float32r if cfg["f32r"] else mybir.dt.float32
    f32 = mybir.dt.float32
    bf16 = mybir.dt.bfloat16
    BT = Bc * Tc
    MQ = hpc * HD                 # 128
    assert MQ == 128
    KT_C = Cc // 128              # 8
    TOKC = 512
    NCH = BT // TOKC              # 8
    QC = Tc // TOKC               # 4
    KTT = Tc // 128               # 16
    MO = Cc // 128                # 8
    CH_PER_B = Tc // TOKC         # 4
    DKT = TOKC // 128             # 4

    nc = bacc.Bacc()
    xT = nc.declare_dram_parameter("xT", [Cc, BT], bf16, isOutput=False)
    wqkvT = nc.declare_dram_parameter("wqkvT", [Cc, 3 * MQ], bf16, isOutput=False)
    bqkv = nc.declare_dram_parameter("bqkv", [3 * MQ, 1], f32, isOutput=False)
    wpT = nc.declare_dram_parameter("wpT", [MQ, Cc], bf16, isOutput=False)
    outT = nc.declare_dram_parameter("outT", [Cc, BT], bf16, isOutput=True)

    xT_r = xT.rearrange("(kt p) t -> p kt t", p=128)
    wq_r = wqkvT.rearrange("(kt p) m -> p kt m", p=128)
    bq_r = bqkv.rearrange("(g p) o -> p (g o)", p=128)

    AF = mybir.ActivationFunctionType

    with tile.TileContext(nc) as tc:
        with (
            tc.tile_pool(name="consts", bufs=1) as consts,
            tc.tile_pool(name="xpool", bufs=12) as xpool,
            tc.tile_pool(name="epool", bufs=4) as epool,
            tc.tile_pool(name="npool", bufs=2) as npool,
            tc.tile_pool(name="ypool", bufs=2) as ypool,
            tc.tile_pool(name="opool", bufs=4) as opool,
            tc.tile_pool(name="ps_x", bufs=2, space="PSUM") as ps_x,
            tc.tile_pool(name="ps_s", bufs=2, space="PSUM") as ps_s,
            tc.tile_pool(name="ps_y", bufs=1, space="PSUM") as ps_y,
        ):
            # ---- const DMAs, doorbells spread over the idle compute
            # queues (sync alone issues one doorbell per ~700ns, which
            # was the whole startup latency) ----
            dmaq = [nc.scalar, nc.gpsimd, nc.sync]
            # critical-path first: interleave (w_kt, x0_kt) across the three
            # DMA queues so the kt=0 pieces are at the head of different
            # queues -- the first QKV matmul needs only w0 + x0_0; b lands
            # before the first bias evict needs it.
            x_tiles = {}
            w_t = []
            b_sb = consts.tile([128, 3], f32, tag="b")
            wp_sb = consts.tile([128, Cc], bf16, tag="wp")
            for kt in range(KT_C):
                w = consts.tile([128, 3 * MQ], bf16, tag=f"w{kt}", name=f"w{kt}")
                dmaq[(2 * kt) % 3].dma_start(out=w, in_=wq_r[:, kt, :])
                w_t.append(w)
                if kt == 3:
                    nc.gpsimd.dma_start(out=b_sb, in_=bq_r)
                x = xpool.tile([128, TOKC], bf16, tag="x", name=f"x0_{kt}",
                               bufs=8)
                dmaq[(2 * kt + 1) % 3].dma_start(out=x, in_=xT_r[:, kt, 0:TOKC])
                x_tiles[(0, kt)] = x
            nc.gpsimd.dma_start(out=wp_sb, in_=wpT[:, :])

            qT_sb = consts.tile([128, BT], bf16, tag="qT")
            kT_sb = consts.tile([128, BT], bf16, tag="kT")
            vT_sb = consts.tile([128, BT], bf16, tag="vT")

            ident = consts.tile([128, 128], f32, tag="ident")
            make_identity(nc, ident)
            ident_bf = consts.tile([128, 128], bf16, tag="ident_bf")
            nc.vector.tensor_copy(ident_bf[:, :], ident[:, :])

            v_sb = [
                consts.tile([128, KTT, hpc, 65], bf16, tag=f"v{b}",
                            name=f"v{b}") for b in range(Bc)
            ]
            for b in range(Bc):
                nc.vector.memset(v_sb[b][:, :, :, 64:65], 1.0)

            # chunks 1-7: half-chunk slabs, fully resident (bufs=8 -> no
            # doorbell ever blocks on x-pool rotation) and spread across
            # all three DMA-capable queues for aggregate HBM bandwidth at
            # startup. Only HWDGE/SWDGE queues (scalar/gpsimd/sync) can
            # issue DMAs.
            slabq = {1: nc.sync, 2: nc.sync, 3: nc.sync, 4: nc.sync,
                     5: nc.sync, 6: nc.sync, 7: nc.sync}
            for ch in range(1, NCH):
                for half in range(2):
                    xh = xpool.tile([128, KT_C // 2, TOKC], bf16, tag="xh",
                                    name=f"x{ch}_h{half}", bufs=8)
                    slabq[ch].dma_start(
                        out=xh,
                        in_=xT_r[:, half * (KT_C // 2):(half + 1) * (KT_C // 2),
                                 ch * TOKC:(ch + 1) * TOKC])
                    for kt in range(KT_C // 2):
                        x_tiles[(ch, half * (KT_C // 2) + kt)] = xh[:, kt, :]

            # ---- filler machinery ----
            # pe_q: ordered (marker, thunk) list — QKV chunks, V transposes.
            # pr_q: (ready_iter, thunk) list — projection work of done groups.
            pe_q = []
            pr_q = []
            iter_ctr = [0]

            def pop_fillers(pr_budget=2, pe_budget=1):
                n = 0
                while pr_q and n < pr_budget and pr_q[0][0] <= iter_ctr[0]:
                    pr_q.pop(0)[1]()
                    n += 1
                n = 0
                while pe_q and n < pe_budget:
                    pe_q.pop(0)[1]()
                    n += 1

            def flush_until(marker):
                while pe_q and pe_q[0][0] <= marker:
                    pe_q.pop(0)[1]()

            def flush_all():
                while pe_q:
                    pe_q.pop(0)[1]()
                while pr_q:
                    pr_q.pop(0)[1]()

            # ---- building blocks ----
            def qkv_mm_group(ch, m):
                ps = ps_x.tile([128, TOKC], f32, tag="mm")
                for kt in range(KT_C):
                    MM_LOG.append(f"qkv c{ch} m{m} k{kt}")
                    nc.tensor.matmul(
                        ps[:, :],
                        w_t[kt][:, m * MQ:(m + 1) * MQ],
                        x_tiles[(ch, kt)],
                        start=(kt == 0), stop=(kt == KT_C - 1),
                    )
                dst = (qT_sb, kT_sb, vT_sb)[m]
                nc.vector.tensor_scalar_add(
                    out=dst[:, ch * TOKC:(ch + 1) * TOKC], in0=ps[:, :],
                    scalar1=b_sb[:, m:m + 1],
                )

            def vtrans_tile(b, kt):
                ps_t = ps_x.tile([128, 128], bf16, tag="mm")
                MM_LOG.append(f"vtrans b{b} k{kt}")
                nc.tensor.transpose(
                    ps_t[:, :],
                    vT_sb[:, b * Tc + kt * 128:b * Tc + (kt + 1) * 128],
                    ident_bf[:, :],
                )
                # split the PSUM->SBUF copies across DVE and ACT so a busy
                # DVE queue cannot starve the v_sb -> AV dependency chain
                nc.vector.tensor_copy(v_sb[b][:, kt, 0, 0:64],
                                      ps_t[:, 0:HD])
                nc.scalar.copy(v_sb[b][:, kt, 1, 0:64],
                               ps_t[:, HD:2 * HD])

            outT_r = outT.rearrange("(mg p) t -> p mg t", p=128)

            def proj_quad(b, qc, yT_t, mo, last=False):
                # mo..mo+3 in one thunk: four matmuls, four evicts into one
                # tile, ONE output DMA doorbell. For the last group (no
                # following PE work to overlap) split evicts across DVE and
                # the then-idle ACT engine to shorten the tail.
                q_sl = slice(b * Tc + qc * TOKC, b * Tc + (qc + 1) * TOKC)
                o_t = opool.tile([128, 4, TOKC], bf16, tag="o")
                for j in range(4):
                    pso = ps_x.tile([128, TOKC], f32, tag="mm")
                    MM_LOG.append(f"proj b{b} q{qc} m{mo + j}")
                    nc.tensor.matmul(
                        pso[:, :],
                        wp_sb[:, (mo + j) * 128:(mo + j + 1) * 128],
                        yT_t[:, :],
                        start=True, stop=True,
                    )
                    if last and j % 2 == 1:
                        nc.scalar.copy(o_t[:, j, :], pso[:, :])
                    else:
                        nc.vector.tensor_copy(o_t[:, j, :], pso[:, :])
                nc.sync.dma_start(
                    out=outT_r[:, mo:mo + 4, q_sl], in_=o_t[:, :, :, ])

            def push_norm_proj(b, qc, psy, delay=0, last=False):
                # deferred normalize: runs as filler inside the NEXT group's
                # k-loop so the serial copy/recip/broadcast/mul chain never
                # blocks the v-evict -> vtrans -> S critical path at group
                # boundaries.
                yT_t = ypool.tile([128, TOKC], bf16, tag="yT")

                def norm_a():
                    # Z copy on ACT (closer to PSUM; keeps the 1-partition
                    # 1.2us op off the congested DVE queue)
                    zrow = npool.tile([1, hpc * TOKC], f32, tag="z")
                    nc.scalar.copy(zrow[:, :], psy[64:65, :, :])
                    rc = npool.tile([1, hpc * TOKC], f32, tag="rc")
                    nc.vector.reciprocal_approx_fast(rc[:, :], zrow[:, :])
                    norm_state[0] = rc

                def norm_bc(hh):
                    # per-head broadcast+mul so GpSimd (bcast) pipelines
                    # against DVE (mul of the other head)
                    rc = norm_state[0]
                    rcb = npool.tile([64, TOKC], f32, tag="rcb")
                    nc.gpsimd.partition_broadcast(
                        rcb[:, :], rc[:, hh * TOKC:(hh + 1) * TOKC])
                    nc.vector.tensor_mul(
                        yT_t[hh * HD:(hh + 1) * HD, :],
                        psy[0:HD, hh, :], rcb[:, :],
                    )

                def norm_last(hh):
                    # last group: per-head zrow/recip/bcast/mul chain so the
                    # ACT/DVE/GpSimd stages of the two heads pipeline --
                    # nothing overlaps the final norm, so latency is all
                    # that matters.
                    zrow = npool.tile([1, TOKC], f32, tag="z")
                    nc.scalar.copy(zrow[:, :], psy[64:65, hh, :])
                    rc = npool.tile([1, TOKC], f32, tag="rc")
                    nc.vector.reciprocal_approx_fast(rc[:, :], zrow[:, :])
                    rcb = npool.tile([64, TOKC], f32, tag="rcb")
                    nc.gpsimd.partition_broadcast(rcb[:, :], rc[:, :])
                    nc.vector.tensor_mul(
                        yT_t[hh * HD:(hh + 1) * HD, :],
                        psy[0:HD, hh, :], rcb[:, :],
                    )

                norm_state = [None]
                if last:
                    pr_q.append((iter_ctr[0] + delay, lambda: norm_last(0)))
                    pr_q.append((iter_ctr[0] + delay, lambda: norm_last(1)))
                else:
                    pr_q.append((iter_ctr[0] + delay, norm_a))
                    pr_q.append((iter_ctr[0] + delay, lambda: norm_bc(0)))
                    pr_q.append((iter_ctr[0] + delay + 1, lambda: norm_bc(1)))
                # reserve the 2nd-to-last group's later quad for the final
                # group's norm window, where the PE otherwise runs dry
                # (only qc==QC-2 is safe: an earlier group's unready quad at
                # the pr_q head would block the NEXT group's norm pops)
                qdel = [delay + 4, delay + 28 if (b == 1 and qc == QC - 2)
                        else delay + 7]
                for i, mo in enumerate(range(0, MO, 4)):
                    pr_q.append((
                        iter_ctr[0] + qdel[i],
                        lambda mo=mo: proj_quad(b, qc, yT_t, mo, last=last),
                    ))

            DEPTH = 2

            def emit_group(b, qc):
                n_kt = (qc + 1) * DKT
                if qc == 0:
                    kts = list(range(DKT))
                else:
                    nd = list(range(0, qc * DKT))
                    dg = list(range(qc * DKT, qc * DKT + DKT))
                    kts = nd[:2] + dg + nd[2:]
                q_sl = slice(b * Tc + qc * TOKC, b * Tc + (qc + 1) * TOKC)
                psy = ps_y.tile([65, hpc, TOKC], f32, tag="y")
                pend = []
                n_av = 0

                def emit_av(kt, e_t):
                    nonlocal n_av
                    q0 = max(0, kt - qc * DKT) * 128
                    for hh in range(hpc):
                        MM_LOG.append(f"av b{b} q{qc} k{kt} h{hh}")
                        nc.tensor.matmul(
                            psy[:, hh, q0:],
                            v_sb[b][:, kt, hh, :],
                            e_t[:, hh, q0:],
                            start=(n_av == 0), stop=(n_av == n_kt - 1),
                        )
                    n_av += 1

                for kt in kts:
                    di = kt - qc * DKT
                    # causal rectangle: for diagonal k-tiles only q >= di*128
                    # can attend -- skip the fully-masked left part of the
                    # S matmul, exp, mask, and AV (their AV slices match).
                    q0 = max(0, di) * 128
                    pss = ps_s.tile([128, hpc, TOKC], f32, tag="s")
                    for hh in range(hpc):
                        # disjoint PE row groups -> the two S matmuls
                        # overlap in the array
                        MM_LOG.append(f"S b{b} q{qc} k{kt} h{hh}")
                        nc.tensor.matmul(
                            pss[:, hh, q0:],
                            kT_sb[hh * HD:(hh + 1) * HD,
                                  b * Tc + kt * 128:b * Tc + (kt + 1) * 128],
                            qT_sb[hh * HD:(hh + 1) * HD,
                                  b * Tc + qc * TOKC + q0:
                                  b * Tc + (qc + 1) * TOKC],
                            start=True, stop=True,
                        )
                    e_t = epool.tile([128, hpc, TOKC], bf16, tag="e")
                    nc.scalar.activation(out=e_t[:, :, q0:],
                                         in_=pss[:, :, q0:],
                                         func=AF.Exp, scale=0.125)
                    if di >= 0:
                        # keep iff q >= k iff (q0 + f) >= p + 128*di iff
                        # f >= p (q0 == 128*di), same for both head halves
                        nc.gpsimd.affine_select(
                            out=e_t[:, :, q0:], in_=e_t[:, :, q0:],
                            compare_op=mybir.AluOpType.is_ge,
                            fill=0.0,
                            base=0,
                            pattern=[[0, hpc], [1, TOKC - q0]],
                            channel_multiplier=-1,
                        )
                    pend.append((kt, e_t))
                    pop_fillers(pr_budget=2, pe_budget=1)
                    iter_ctr[0] += 1
                    if len(pend) > DEPTH:
                        emit_av(*pend.pop(0))
                for kt, e_t in pend:
                    emit_av(kt, e_t)
                return psy

            # ---- schedule ----
            # v (m=2) first within each chunk so its DVE evict and the
            # dependent V transposes land as early as possible -- AV of the
            # next group's diagonal tiles was stalling ~4us on late vtrans.
            for m in (2, 0, 1):
                qkv_mm_group(0, m)
            for kt in range(DKT):
                vtrans_tile(0, kt)

            # filler inventory with ordering markers: marker value gates
            # correctness flushes before dependent attention groups
            def push_pe(marker, thunk):
                pe_q.append((marker, thunk))

            for ch in range(1, CH_PER_B + 2):        # ch 1..5
                push_pe(ch, lambda ch=ch: qkv_mm_group(ch, 2))
                if ch < CH_PER_B:                    # b0 transposes kt 4..15
                    for kt in range(ch * DKT, (ch + 1) * DKT):
                        push_pe(ch, lambda kt=kt: vtrans_tile(0, kt))
                for m in (0, 1):
                    push_pe(ch, lambda ch=ch, m=m: qkv_mm_group(ch, m))

            for qc in range(QC):                     # b0 attention
                flush_until(qc)                      # chunks/trans <= qc
                psy = emit_group(0, qc)
                push_norm_proj(0, qc, psy)

            # remaining b1 inventory: trans for ch4/5, then ch6/7 + trans
            for kt in range(0, 2 * DKT):
                push_pe(10, lambda kt=kt: vtrans_tile(1, kt))
            for ch in range(CH_PER_B + 2, NCH):      # ch 6, 7
                push_pe(ch + 5, lambda ch=ch: qkv_mm_group(ch, 2))
                for kt in range((ch - CH_PER_B) * DKT,
                                (ch - CH_PER_B + 1) * DKT):
                    push_pe(ch + 5, lambda kt=kt: vtrans_tile(1, kt))
                for m in (0, 1):
                    push_pe(ch + 5, lambda ch=ch, m=m: qkv_mm_group(ch, m))

            for qc in range(QC):                     # b1 attention
                flush_until(10 + qc)
                psy = emit_group(1, qc)
                push_norm_proj(1, qc, psy, last=(qc == QC - 1))
            flush_all()

    nc.finalize()
    return nc


def prep_inputs(cfg, x, W_attn, b_attn, W_proj, b_proj):
    """Host-side sharding: returns per-core input dicts."""
    Bc, Tc, Cc, hpc = cfg["B"], cfg["T"], cfg["C"], cfg["HPC"]
    n_cores = (Cc // HD) // hpc
    BT = Bc * Tc
    MQ = hpc * HD

    import ml_dtypes
    x = np.ascontiguousarray(x, dtype=np.float32)
    xT = np.ascontiguousarray(x.reshape(BT, Cc).T).astype(ml_dtypes.bfloat16)

    in_maps = []
    for c in range(n_cores):
        r0 = c * MQ
        rows = []
        for g in range(3):
            rows.append(np.arange(g * Cc + r0, g * Cc + r0 + MQ))
        rows = np.concatenate(rows)
        w_slice = W_attn[rows, :]                       # [384, C]
        wqkvT = np.ascontiguousarray(w_slice.T)         # [C, 384]
        bq = np.ascontiguousarray(b_attn[rows].reshape(MQ * 3, 1))
        wpT = np.ascontiguousarray(W_proj[:, r0:r0 + MQ].T).astype(ml_dtypes.bfloat16)
        in_maps.append({
            "xT": xT,
            "wqkvT": wqkvT.astype(ml_dtypes.bfloat16),
            "bqkv": bq.astype(np.float32),
            "wpT": wpT,
        })
    return in_maps


def combine(cfg, results, b_proj):
    Bc, Tc, Cc = cfg["B"], cfg["T"], cfg["C"]
    acc = results[0]["outT"].astype(np.float32)
    for r in results[1:]:
        acc = acc + r["outT"].astype(np.float32)
    out = acc.T + b_proj[None, :]
    return np.ascontiguousarray(out.reshape(Bc, Tc, Cc).astype(np.float32))


_NC_CACHE = {}


def kernel(x, W_attn, b_attn, W_proj, b_proj):
    from concourse.bass_utils import run_bass_kernel_spmd

    cfg = _cfg_full()
    key = "full"
    if key not in _NC_CACHE:
        _NC_CACHE[key] = build_nc(cfg)
    nc = _NC_CACHE[key]
    in_maps = prep_inputs(cfg, np.asarray(x), np.asarray(W_attn),
                          np.asarray(b_attn), np.asarray(W_proj),
                          np.asarray(b_proj))
    res = run_bass_kernel_spmd(nc, in_maps, list(range(N_CORES)))
    return combine(cfg, res.results, np.asarray(b_proj, dtype=np.float32))



# revision 49
# speedup vs baseline: 1.0148x; 1.0148x over previous
"""Causal self-attention Trainium2 kernel (8-core head-parallel tensor parallel).

v5 strategy (per core, 2 heads, feature-major dataflow; all-bf16 matmul
operands -- fp8 fails the 2e-2 gate on this data, fp32 moving operands
cost 2x on the PE's moving-data path):
  - QKV: qkv^T = W^T.T @ x^T per 512-token chunk; x and W are host-cast
    to bf16 so each matmul streams at the 216ns/512-col floor. v (m=2)
    is computed FIRST in each chunk so its DVE bias-evict and the
    dependent V transposes never gate the next group's diagonal AV.
    Chunk 0 runs up front; later chunks ride as PE filler inside the
    attention stream.
  - Attention per (b, q-chunk) group, k-tile loop with BOTH heads per
    step: S pair into one 2-bank PSUM tile -> ONE exp on ACT per step
    -> causal affine_select on GpSimd for diagonal tiles. Diagonal
    k-tiles use causal RECTANGLES: S/exp/mask/AV all skip the fully
    masked q < 128*di slice (saves ~20% of attention work and exp).
  - Z rides as a ones-column in V (AV row 64). Normalize is DEFERRED:
    pushed into the next group's k-loop via the filler queue (Z copy on
    ACT, reciprocal_approx_fast on DVE, per-head partition_broadcast on
    GpSimd + multiply on DVE), so the serial chain never blocks the
    v-evict -> vtrans -> S critical path at group boundaries.
  - PE never idles: a filler queue (later QKV chunks, V transposes) and
    a deferred queue (norm + projection of completed groups) are popped
    every k-tile step; one projection quad is held back for the final
    group's norm window. Projection output DMAs are 4-chunk batched.
  - Host: sum 8 bf16 partial out^T in f32, transpose, +b_proj.
"""

import sys

if "/opt/trn_rl_repo" not in sys.path:
    sys.path.insert(0, "/opt/trn_rl_repo")

import numpy as np

# ---- problem constants (hardcoded for the grading harness) ----
B, T, C, H = 2, 2048, 1024, 16
HD = C // H            # 64
N_CORES = 8
HPC = H // N_CORES     # heads per core = 2

_F32R = True


def _cfg_full():
    return dict(B=B, T=T, C=C, HPC=HPC, f32r=_F32R)


MM_LOG = []


def build_nc(cfg):
    """Build the single-core SPMD Bass program."""
    import concourse.bacc as bacc
    import concourse.mybir as mybir
    import concourse.tile as tile
    from concourse.masks import make_identity

    MM_LOG.clear()

    Bc, Tc, Cc, hpc = cfg["B"], cfg["T"], cfg["C"], cfg["HPC"]
    f32r = mybir.dt.float32r if cfg["f32r"] else mybir.dt.float32
    f32 = mybir.dt.float32
    bf16 = mybir.dt.bfloat16
    BT = Bc * Tc
    MQ = hpc * HD                 # 128
    assert MQ == 128
    KT_C = Cc // 128              # 8
    TOKC = 512
    NCH = BT // TOKC              # 8
    QC = Tc // TOKC               # 4
    KTT = Tc // 128               # 16
    MO = Cc // 128                # 8
    CH_PER_B = Tc // TOKC         # 4
    DKT = TOKC // 128             # 4

    nc = bacc.Bacc()
    xT = nc.declare_dram_parameter("xT", [Cc, BT], bf16, isOutput=False)
    wqkvT = nc.declare_dram_parameter("wqkvT", [Cc, 3 * MQ], bf16, isOutput=False)
    bqkv = nc.declare_dram_parameter("bqkv", [3 * MQ, 1], f32, isOutput=False)
    wpT = nc.declare_dram_parameter("wpT", [MQ, Cc], bf16, isOutput=False)
    outT = nc.declare_dram_parameter("outT", [Cc, BT], bf16, isOutput=True)

    xT_r = xT.rearrange("(kt p) t -> p kt t", p=128)
    wq_r = wqkvT.rearrange("(kt p) m -> p kt m", p=128)
    bq_r = bqkv.rearrange("(g p) o -> p (g o)", p=128)

    AF = mybir.ActivationFunctionType

    with tile.TileContext(nc) as tc:
        with (
            tc.tile_pool(name="consts", bufs=1) as consts,
            tc.tile_pool(name="xpool", bufs=12) as xpool,
            tc.tile_pool(name="epool", bufs=6) as epool,
            tc.tile_pool(name="npool", bufs=3) as npool,
            tc.tile_pool(name="ypool", bufs=3) as ypool,
            tc.tile_pool(name="opool", bufs=6) as opool,
            tc.tile_pool(name="ps_x", bufs=2, space="PSUM") as ps_x,
            tc.tile_pool(name="ps_s", bufs=2, space="PSUM") as ps_s,
            tc.tile_pool(name="ps_y", bufs=1, space="PSUM") as ps_y,
        ):
            # ---- const DMAs, doorbells spread over the idle compute
            # queues (sync alone issues one doorbell per ~700ns, which
            # was the whole startup latency) ----
            dmaq = [nc.scalar, nc.gpsimd, nc.sync]
            # critical-path first: interleave (w_kt, x0_kt) across the three
            # DMA queues so the kt=0 pieces are at the head of different
            # queues -- the first QKV matmul needs only w0 + x0_0; b lands
            # before the first bias evict needs it.
            x_tiles = {}
            w_t = []
            b_sb = consts.tile([128, 3], f32, tag="b")
            wp_sb = consts.tile([128, Cc], bf16, tag="wp")
            for kt in range(KT_C):
                w = consts.tile([128, 3 * MQ], bf16, tag=f"w{kt}", name=f"w{kt}")
                dmaq[(2 * kt) % 3].dma_start(out=w, in_=wq_r[:, kt, :])
                w_t.append(w)
                if kt == 3:
                    nc.gpsimd.dma_start(out=b_sb, in_=bq_r)
                x = xpool.tile([128, TOKC], bf16, tag="x", name=f"x0_{kt}",
                               bufs=8)
                dmaq[(2 * kt + 1) % 3].dma_start(out=x, in_=xT_r[:, kt, 0:TOKC])
                x_tiles[(0, kt)] = x
            nc.gpsimd.dma_start(out=wp_sb, in_=wpT[:, :])

            qT_sb = consts.tile([128, BT], bf16, tag="qT")
            kT_sb = consts.tile([128, BT], bf16, tag="kT")
            vT_sb = consts.tile([128, BT], bf16, tag="vT")

            ident = consts.tile([128, 128], f32, tag="ident")
            make_identity(nc, ident)
            ident_bf = consts.tile([128, 128], bf16, tag="ident_bf")
            nc.vector.tensor_copy(ident_bf[:, :], ident[:, :])

            v_sb = [
                consts.tile([128, KTT, hpc, 65], bf16, tag=f"v{b}",
                            name=f"v{b}") for b in range(Bc)
            ]
            for b in range(Bc):
                nc.vector.memset(v_sb[b][:, :, :, 64:65], 1.0)

            # chunks 1-7: half-chunk slabs, fully resident (bufs=8 -> no
            # doorbell ever blocks on x-pool rotation) and spread across
            # all three DMA-capable queues for aggregate HBM bandwidth at
            # startup. Only HWDGE/SWDGE queues (scalar/gpsimd/sync) can
            # issue DMAs.
            slabq = {1: nc.sync, 2: nc.sync, 3: nc.sync, 4: nc.sync,
                     5: nc.sync, 6: nc.sync, 7: nc.sync}
            for ch in range(1, NCH):
                for half in range(2):
                    xh = xpool.tile([128, KT_C // 2, TOKC], bf16, tag="xh",
                                    name=f"x{ch}_h{half}", bufs=8)
                    slabq[ch].dma_start(
                        out=xh,
                        in_=xT_r[:, half * (KT_C // 2):(half + 1) * (KT_C // 2),
                                 ch * TOKC:(ch + 1) * TOKC])
                    for kt in range(KT_C // 2):
                        x_tiles[(ch, half * (KT_C // 2) + kt)] = xh[:, kt, :]

            # ---- filler machinery ----
            # pe_q: ordered (marker, thunk) list — QKV chunks, V transposes.
            # pr_q: (ready_iter, thunk) list — projection work of done groups.
            pe_q = []
            pr_q = []
            iter_ctr = [0]

            def pop_fillers(pr_budget=2, pe_budget=1):
                n = 0
                while pr_q and n < pr_budget and pr_q[0][0] <= iter_ctr[0]:
                    pr_q.pop(0)[1]()
                    n += 1
                n = 0
                while pe_q and n < pe_budget:
                    pe_q.pop(0)[1]()
                    n += 1

            def flush_until(marker):
                while pe_q and pe_q[0][0] <= marker:
                    pe_q.pop(0)[1]()

            def flush_all():
                while pe_q:
                    pe_q.pop(0)[1]()
                while pr_q:
                    pr_q.pop(0)[1]()

            # ---- building blocks ----
            def qkv_mm_group(ch, m):
                ps = ps_x.tile([128, TOKC], f32, tag="mm")
                for kt in range(KT_C):
                    MM_LOG.append(f"qkv c{ch} m{m} k{kt}")
                    nc.tensor.matmul(
                        ps[:, :],
                        w_t[kt][:, m * MQ:(m + 1) * MQ],
                        x_tiles[(ch, kt)],
                        start=(kt == 0), stop=(kt == KT_C - 1),
                    )
                dst = (qT_sb, kT_sb, vT_sb)[m]
                nc.vector.tensor_scalar_add(
                    out=dst[:, ch * TOKC:(ch + 1) * TOKC], in0=ps[:, :],
                    scalar1=b_sb[:, m:m + 1],
                )

            def vtrans_tile(b, kt):
                ps_t = ps_x.tile([128, 128], bf16, tag="mm")
                MM_LOG.append(f"vtrans b{b} k{kt}")
                nc.tensor.transpose(
                    ps_t[:, :],
                    vT_sb[:, b * Tc + kt * 128:b * Tc + (kt + 1) * 128],
                    ident_bf[:, :],
                )
                # split the PSUM->SBUF copies across DVE and ACT so a busy
                # DVE queue cannot starve the v_sb -> AV dependency chain
                nc.vector.tensor_copy(v_sb[b][:, kt, 0, 0:64],
                                      ps_t[:, 0:HD])
                nc.scalar.copy(v_sb[b][:, kt, 1, 0:64],
                               ps_t[:, HD:2 * HD])

            outT_r = outT.rearrange("(mg p) t -> p mg t", p=128)

            def proj_quad(b, qc, yT_t, mo, last=False):
                # mo..mo+3 in one thunk: four matmuls, four evicts into one
                # tile, ONE output DMA doorbell. For the last group (no
                # following PE work to overlap) split evicts across DVE and
                # the then-idle ACT engine to shorten the tail.
                q_sl = slice(b * Tc + qc * TOKC, b * Tc + (qc + 1) * TOKC)
                o_t = opool.tile([128, 4, TOKC], bf16, tag="o")
                for j in range(4):
                    pso = ps_x.tile([128, TOKC], f32, tag="mm")
                    MM_LOG.append(f"proj b{b} q{qc} m{mo + j}")
                    nc.tensor.matmul(
                        pso[:, :],
                        wp_sb[:, (mo + j) * 128:(mo + j + 1) * 128],
                        yT_t[:, :],
                        start=True, stop=True,
                    )
                    if last and j % 2 == 1:
                        nc.scalar.copy(o_t[:, j, :], pso[:, :])
                    else:
                        nc.vector.tensor_copy(o_t[:, j, :], pso[:, :])
                nc.sync.dma_start(
                    out=outT_r[:, mo:mo + 4, q_sl], in_=o_t[:, :, :, ])

            def push_norm_proj(b, qc, psy, delay=0, last=False):
                # deferred normalize: runs as filler inside the NEXT group's
                # k-loop so the serial copy/recip/broadcast/mul chain never
                # blocks the v-evict -> vtrans -> S critical path at group
                # boundaries.
                yT_t = ypool.tile([128, TOKC], bf16, tag="yT")

                def norm_a():
                    # Z copy on ACT (closer to PSUM; keeps the 1-partition
                    # 1.2us op off the congested DVE queue)
                    zrow = npool.tile([1, hpc * TOKC], f32, tag="z")
                    nc.scalar.copy(zrow[:, :], psy[64:65, :, :])
                    rc = npool.tile([1, hpc * TOKC], f32, tag="rc")
                    nc.vector.reciprocal_approx_fast(rc[:, :], zrow[:, :])
                    norm_state[0] = rc

                def norm_bc(hh):
                    # per-head broadcast+mul so GpSimd (bcast) pipelines
                    # against DVE (mul of the other head)
                    rc = norm_state[0]
                    rcb = npool.tile([64, TOKC], f32, tag="rcb")
                    nc.gpsimd.partition_broadcast(
                        rcb[:, :], rc[:, hh * TOKC:(hh + 1) * TOKC])
                    nc.vector.tensor_mul(
                        yT_t[hh * HD:(hh + 1) * HD, :],
                        psy[0:HD, hh, :], rcb[:, :],
                    )

                def norm_last(hh):
                    # last group: per-head zrow/recip/bcast/mul chain so the
                    # ACT/DVE/GpSimd stages of the two heads pipeline --
                    # nothing overlaps the final norm, so latency is all
                    # that matters.
                    zrow = npool.tile([1, TOKC], f32, tag="z")
                    nc.scalar.copy(zrow[:, :], psy[64:65, hh, :])
                    rc = npool.tile([1, TOKC], f32, tag="rc")
                    nc.vector.reciprocal_approx_fast(rc[:, :], zrow[:, :])
                    rcb = npool.tile([64, TOKC], f32, tag="rcb")
                    nc.gpsimd.partition_broadcast(rcb[:, :], rc[:, :])
                    nc.vector.tensor_mul(
                        yT_t[hh * HD:(hh + 1) * HD, :],
                        psy[0:HD, hh, :], rcb[:, :],
                    )

                norm_state = [None]
                if last:
                    pr_q.append((iter_ctr[0] + delay, lambda: norm_last(0)))
                    pr_q.append((iter_ctr[0] + delay, lambda: norm_last(1)))
                else:
                    pr_q.append((iter_ctr[0] + delay, norm_a))
                    pr_q.append((iter_ctr[0] + delay, lambda: norm_bc(0)))
                    pr_q.append((iter_ctr[0] + delay + 1, lambda: norm_bc(1)))
                # reserve the 2nd-to-last group's later quad for the final
                # group's norm window, where the PE otherwise runs dry
                # (only qc==QC-2 is safe: an earlier group's unready quad at
                # the pr_q head would block the NEXT group's norm pops)
                qdel = [delay + 4, delay + 28 if (b == 1 and qc == QC - 2)
                        else delay + 7]
                for i, mo in enumerate(range(0, MO, 4)):
                    pr_q.append((
                        iter_ctr[0] + qdel[i],
                        lambda mo=mo: proj_quad(b, qc, yT_t, mo, last=last),
                    ))

            DEPTH = 3

            def emit_group(b, qc):
                n_kt = (qc + 1) * DKT
                if qc == 0:
                    kts = list(range(DKT))
                else:
                    nd = list(range(0, qc * DKT))
                    dg = list(range(qc * DKT, qc * DKT + DKT))
                    kts = nd[:2] + dg + nd[2:]
                q_sl = slice(b * Tc + qc * TOKC, b * Tc + (qc + 1) * TOKC)
                psy = ps_y.tile([65, hpc, TOKC], f32, tag="y")
                pend = []
                n_av = 0

                def emit_av(kt, e_t):
                    nonlocal n_av
                    q0 = max(0, kt - qc * DKT) * 128
                    for hh in range(hpc):
                        MM_LOG.append(f"av b{b} q{qc} k{kt} h{hh}")
                        nc.tensor.matmul(
                            psy[:, hh, q0:],
                            v_sb[b][:, kt, hh, :],
                            e_t[:, hh, q0:],
                            start=(n_av == 0), stop=(n_av == n_kt - 1),
                        )
                    n_av += 1

                for kt in kts:
                    di = kt - qc * DKT
                    # causal rectangle: for diagonal k-tiles only q >= di*128
                    # can attend -- skip the fully-masked left part of the
                    # S matmul, exp, mask, and AV (their AV slices match).
                    q0 = max(0, di) * 128
                    pss = ps_s.tile([128, hpc, TOKC], f32, tag="s")
                    for hh in range(hpc):
                        # disjoint PE row groups -> the two S matmuls
                        # overlap in the array
                        MM_LOG.append(f"S b{b} q{qc} k{kt} h{hh}")
                        nc.tensor.matmul(
                            pss[:, hh, q0:],
                            kT_sb[hh * HD:(hh + 1) * HD,
                                  b * Tc + kt * 128:b * Tc + (kt + 1) * 128],
                            qT_sb[hh * HD:(hh + 1) * HD,
                                  b * Tc + qc * TOKC + q0:
                                  b * Tc + (qc + 1) * TOKC],
                            start=True, stop=True,
                        )
                    e_t = epool.tile([128, hpc, TOKC], bf16, tag="e")
                    nc.scalar.activation(out=e_t[:, :, q0:],
                                         in_=pss[:, :, q0:],
                                         func=AF.Exp, scale=0.125)
                    if di >= 0:
                        # keep iff q >= k iff (q0 + f) >= p + 128*di iff
                        # f >= p (q0 == 128*di), same for both head halves
                        nc.gpsimd.affine_select(
                            out=e_t[:, :, q0:], in_=e_t[:, :, q0:],
                            compare_op=mybir.AluOpType.is_ge,
                            fill=0.0,
                            base=0,
                            pattern=[[0, hpc], [1, TOKC - q0]],
                            channel_multiplier=-1,
                        )
                    pend.append((kt, e_t))
                    pop_fillers(pr_budget=2, pe_budget=1)
                    iter_ctr[0] += 1
                    if len(pend) > DEPTH:
                        emit_av(*pend.pop(0))
                for kt, e_t in pend:
                    emit_av(kt, e_t)
                return psy

            # ---- schedule ----
            # v (m=2) first within each chunk so its DVE evict and the
            # dependent V transposes land as early as possible -- AV of the
            # next group's diagonal tiles was stalling ~4us on late vtrans.
            for m in (2, 0, 1):
                qkv_mm_group(0, m)
            for kt in range(DKT):
                vtrans_tile(0, kt)

            # filler inventory with ordering markers: marker value gates
            # correctness flushes before dependent attention groups
            def push_pe(marker, thunk):
                pe_q.append((marker, thunk))

            for ch in range(1, CH_PER_B + 2):        # ch 1..5
                push_pe(ch, lambda ch=ch: qkv_mm_group(ch, 2))
                if ch < CH_PER_B:                    # b0 transposes kt 4..15
                    for kt in range(ch * DKT, (ch + 1) * DKT):
                        push_pe(ch, lambda kt=kt: vtrans_tile(0, kt))
                for m in (0, 1):
                    push_pe(ch, lambda ch=ch, m=m: qkv_mm_group(ch, m))

            for qc in range(QC):                     # b0 attention
                flush_until(qc)                      # chunks/trans <= qc
                psy = emit_group(0, qc)
                push_norm_proj(0, qc, psy)

            # remaining b1 inventory: trans for ch4/5, then ch6/7 + trans
            for kt in range(0, 2 * DKT):
                push_pe(10, lambda kt=kt: vtrans_tile(1, kt))
            for ch in range(CH_PER_B + 2, NCH):      # ch 6, 7
                push_pe(ch + 5, lambda ch=ch: qkv_mm_group(ch, 2))
                for kt in range((ch - CH_PER_B) * DKT,
                                (ch - CH_PER_B + 1) * DKT):
                    push_pe(ch + 5, lambda kt=kt: vtrans_tile(1, kt))
                for m in (0, 1):
                    push_pe(ch + 5, lambda ch=ch, m=m: qkv_mm_group(ch, m))

            for qc in range(QC):                     # b1 attention
                flush_until(10 + qc)
                psy = emit_group(1, qc)
                push_norm_proj(1, qc, psy, last=(qc == QC - 1))
            flush_all()

    nc.finalize()
    return nc


def prep_inputs(cfg, x, W_attn, b_attn, W_proj, b_proj):
    """Host-side sharding: returns per-core input dicts."""
    Bc, Tc, Cc, hpc = cfg["B"], cfg["T"], cfg["C"], cfg["HPC"]
    n_cores = (Cc // HD) // hpc
    BT = Bc * Tc
    MQ = hpc * HD

    import ml_dtypes
    x = np.ascontiguousarray(x, dtype=np.float32)
    xT = np.ascontiguousarray(x.reshape(BT, Cc).T).astype(ml_dtypes.bfloat16)

    in_maps = []
    for c in range(n_cores):
        r0 = c * MQ
        rows = []
        for g in range(3):
            rows.append(np.arange(g * Cc + r0, g * Cc + r0 + MQ))
        rows = np.concatenate(rows)
        w_slice = W_attn[rows, :]                       # [384, C]
        wqkvT = np.ascontiguousarray(w_slice.T)         # [C, 384]
        bq = np.ascontiguousarray(b_attn[rows].reshape(MQ * 3, 1))
        wpT = np.ascontiguousarray(W_proj[:, r0:r0 + MQ].T).astype(ml_dtypes.bfloat16)
        in_maps.append({
            "xT": xT,
            "wqkvT": wqkvT.astype(ml_dtypes.bfloat16),
            "bqkv": bq.astype(np.float32),
            "wpT": wpT,
        })
    return in_maps


def combine(cfg, results, b_proj):
    Bc, Tc, Cc = cfg["B"], cfg["T"], cfg["C"]
    acc = results[0]["outT"].astype(np.float32)
    for r in results[1:]:
        acc = acc + r["outT"].astype(np.float32)
    out = acc.T + b_proj[None, :]
    return np.ascontiguousarray(out.reshape(Bc, Tc, Cc).astype(np.float32))


_NC_CACHE = {}


def kernel(x, W_attn, b_attn, W_proj, b_proj):
    from concourse.bass_utils import run_bass_kernel_spmd

    cfg = _cfg_full()
    key = "full"
    if key not in _NC_CACHE:
        _NC_CACHE[key] = build_nc(cfg)
    nc = _NC_CACHE[key]
    in_maps = prep_inputs(cfg, np.asarray(x), np.asarray(W_attn),
                          np.asarray(b_attn), np.asarray(W_proj),
                          np.asarray(b_proj))
    res = run_bass_kernel_spmd(nc, in_maps, list(range(N_CORES)))
    return combine(cfg, res.results, np.asarray(b_proj, dtype=np.float32))

